# revision 1
# baseline (speedup 1.0000x reference)
"""Causal self-attention (B=4, T=2048, C=1024, H=16) on 8 trn2 NeuronCores.

Sharding: data-parallel over batch (4) x tensor-parallel over heads (2 groups
of 8). Core c handles batch c//2, head-group c%2. Each core computes its
partial output projection (W_proj rows of its heads); the host sums the two
head-group partials per batch and adds b_proj.

Per-core kernel (all matmuls fp32r = full PE rate, ~1e-4 rel err):
  phase 1: QKV projection from streamed x^T chunks. Q^T,K^T stored
           [head-dim, T] (j on partitions), V stored [T, heads, 65] with a
           ones column (65th) so the attention AV matmul produces the softmax
           denominator for free.
  phase 2: attention per head, per 1024-wide q-half, per key-block kb of 128:
           S^T[k,q] = K^T_blk.T @ Q^T (causal: only q >= kb*128), exp on ACT
           (scale=1/sqrt(64) folded in), diag-block mask on DVE, then
           y^T[65, q] += V1_kb.T @ P^T_kb accumulated in PSUM over kb.
           Softmax division after the kb loop: reciprocal of the denominator
           row, broadcast to 128 partitions via a K=1 matmul into the second
           y-psum slot, multiply + evict on DVE.
  phase 3: partial out[t, e] = sum_j y^T[j, t] * W_proj[j, e].

All inputs arrive host-pre-tiled in SBUF layout so every load DMA is a
contiguous DRAM read with >=4KB-per-partition descriptors.
"""

from contextlib import ExitStack

import numpy as np

import concourse.bass as bass
import concourse.mybir as mybir
import concourse.tile as tile
from concourse import bacc
from concourse.bass_utils import run_bass_kernel_spmd
from concourse.masks import make_upper_triangular

P = 128
T = 2048
C = 1024
HG = 8          # heads per core
D = 64
DG = HG * D     # 512
KT = C // P     # 8 contraction tiles for the qkv projection
JT = DG // P    # 4 row-tiles of Q^T/K^T (2 heads each)
TB = T // P     # 16 t/key blocks
QC = T // 512   # 4 512-wide column chunks
HQ = T // 1024  # 2 q-halves in attention
f32 = mybir.dt.float32
f32r = mybir.dt.float32r
EXP = mybir.ActivationFunctionType.Exp


def _attention_head(nc, h, Kt, Qt, V1, Yt, mask_ut, ones65,
                    p_pt, p_tmp, p_dn, ps_s, ps_y):
    j, hp = divmod(h, 2)
    pb = hp * 64                # partition base of this head in Kt/Qt
    for half in range(HQ):
        qlo, qhi = half * 1024, (half + 1) * 1024
        yt_ps = ps_y.tile([65, 1024], f32, tag="y")
        kmax = 8 if half == 0 else TB
        for kb in range(kmax):
            q0 = max(kb * P, qlo)
            plen = qhi - q0
            s_ps = ps_s.tile([P, 1024], f32, tag="s")
            # score chunks, bank-aligned within the piece
            for o in range(0, plen, 512):
                clen = min(512, plen - o)
                nc.tensor.matmul(
                    s_ps[:, o:o + clen],
                    Kt[pb:pb + 64, j, kb * P:(kb + 1) * P],
                    Qt[pb:pb + 64, j, q0 + o:q0 + o + clen],
                    start=True, stop=True)
            pt = p_pt.tile([P, 1024], f32r, tag="pt")
            nc.scalar.activation(pt[:, :plen], s_ps[:, :plen], EXP, scale=0.125)
            if q0 == kb * P:
                # causal mask inside the diagonal 128x128 block
                nc.vector.tensor_mul(pt[:, 0:P], pt[:, 0:P], mask_ut[:])
            # AV accumulate, chunks aligned to the global 512 grid
            c_off = q0
            while c_off < qhi:
                cell = c_off // 512
                c_end = (cell + 1) * 512
                nc.tensor.matmul(
                    yt_ps[0:65, c_off - qlo:c_end - qlo],
                    V1[:, kb, h],
                    pt[:, c_off - q0:c_end - q0],
                    start=(kb == 0), stop=(kb == 4 * cell + 3))
                c_off = c_end
        # denominator -> reciprocal -> broadcast -> scale+evict
        dn = p_dn.tile([65, 1024], f32r, tag="dn")
        with nc.allow_low_precision(reason="softmax 1/denom"):
            nc.vector.reciprocal(dn[64:65, :], yt_ps[64:65, :])
        bc_ps = ps_y.tile([P, 1024], f32, tag="y")
        for m in range(2):
            sl = slice(m * 512, (m + 1) * 512)
            nc.tensor.matmul(bc_ps[:, sl], ones65[64:65, :], dn[64:65, sl],
                             start=True, stop=True)
        sl_t = slice(qlo, qhi)
        if hp == 0:
            dst = Yt[0:64, j, sl_t]
            nc.vector.tensor_copy(dst, yt_ps[0:64, :])
            nc.vector.tensor_mul(dst, dst, bc_ps[0:64, :])
        else:
            y_tmp = p_tmp.tile([64, 1024], f32r, tag="yt")
            nc.vector.tensor_copy(y_tmp[:], yt_ps[0:64, :])
            nc.vector.tensor_mul(y_tmp[:], y_tmp[:], bc_ps[0:64, :])
            nc.gpsimd.dma_start(out=Yt[64:128, j, sl_t], in_=y_tmp[:])


def _emit(nc, tc, rep, xq, wq, wk, wv, wp, bq, bk, bv, out, phases=(1, 2, 3)):
    with ExitStack() as es:
        pfx = f"r{rep}_"
        p_const = es.enter_context(tc.tile_pool(name=pfx + "const", bufs=1))
        mask32 = p_const.tile([P, P], f32)
        make_upper_triangular(nc, mask32[:], val=1.0, diag=True)
        mask_ut = p_const.tile([P, P], f32r)
        nc.vector.tensor_copy(mask_ut[:], mask32[:])
        ones32 = p_const.tile([P, P], f32)
        nc.gpsimd.memset(ones32[:], 1.0)
        ones65 = p_const.tile([65, P], f32r)
        nc.vector.tensor_copy(ones65[:], ones32[0:65, :])
        # keep ACT in the body even in phase-bisected builds (an engine with
        # zero instructions hangs the For_i back-edge barrier)
        act_dummy = p_const.tile([1, 2], f32)
        nc.scalar.copy(act_dummy[:], ones32[0:1, 0:2])

        # persistent tensors
        p_qkv = es.enter_context(tc.tile_pool(name=pfx + "qkv", bufs=1))
        V1 = p_qkv.tile([P, TB, HG, 65], f32r)  # V with ones column, [t, h, d|1]
        Kt = p_qkv.tile([P, JT, T], f32r)       # K^T: [j-dim, T]
        Qt = p_qkv.tile([P, JT, T], f32r)       # Q^T
        # ---------------- phase 1: qkv projection ------------------------
        with ExitStack() as es1:
            p_w1 = es1.enter_context(tc.tile_pool(name=pfx + "w1", bufs=1))
            p_x = es1.enter_context(tc.tile_pool(name=pfx + "x", bufs=2))
            ps1 = es1.enter_context(
                tc.tile_pool(name=pfx + "ps1", bufs=4, space="PSUM"))

            # wv + first x chunk first (first matmuls need them)
            wv_t = p_w1.tile([P, KT, DG], f32r)
            nc.sync.dma_start(out=wv_t[:, 0:2], in_=wv[:, 0:2])
            nc.sync.dma_start(out=wv_t[:, 2:KT], in_=wv[:, 2:KT])
            xc0 = p_x.tile([P, KT, 512], f32r, tag="xc")
            nc.sync.dma_start(out=xc0[:, 0:2], in_=xq[0, :, 0:2])
            nc.sync.dma_start(out=xc0[:, 2:KT], in_=xq[0, :, 2:KT])
            bv_row = p_w1.tile([1, DG], f32r)
            nc.sync.dma_start(out=bv_row[:],
                              in_=bv.rearrange("(o n) -> o n", o=1))
            wk_t = p_w1.tile([P, KT, DG], f32r)
            nc.sync.dma_start(out=wk_t[:], in_=wk[:, :])
            wq_t = p_w1.tile([P, KT, DG], f32r)
            nc.sync.dma_start(out=wq_t[:], in_=wq[:, :])
            bq_t = p_const.tile([P, JT], f32)
            nc.sync.dma_start(out=bq_t[:], in_=bq[:, :])
            bk_t = p_const.tile([P, JT], f32)
            nc.sync.dma_start(out=bk_t[:], in_=bk[:, :])

            # broadcast b_v to 128 partitions (K=1 matmul with a ones row)
            bv_ps = ps1.tile([P, DG], f32, tag="ps")
            nc.tensor.matmul(bv_ps[:], ones65[0:1, :], bv_row[:],
                             start=True, stop=True)
            bias_v = p_w1.tile([P, DG], f32)
            nc.vector.tensor_copy(bias_v[:], bv_ps[:])
            # ones column of V1 (65th dim entry per head)
            nc.vector.tensor_copy(
                V1[:, :, :, 64],
                ones32[:, :].rearrange("p (a b) -> p a b", a=TB))

            for qc in range(QC):
                if qc == 0:
                    xc = xc0
                else:
                    xc = p_x.tile([P, KT, 512], f32r, tag="xc")
                    nc.sync.dma_start(out=xc[:, 0:2], in_=xq[qc, :, 0:2])
                    nc.sync.dma_start(out=xc[:, 2:KT], in_=xq[qc, :, 2:KT])
                # V rows for these 4 t-blocks: out[t, j] = x @ Wv
                for t4 in range(4):
                    tb = qc * 4 + t4
                    ps = ps1.tile([P, DG], f32, tag="ps")
                    for c in range(KT):
                        nc.tensor.matmul(
                            ps[:], xc[:, c, t4 * P:(t4 + 1) * P],
                            wv_t[:, c], start=(c == 0), stop=(c == KT - 1))
                    nc.vector.tensor_add(
                        V1[:, tb, :, 0:64],
                        ps[:].rearrange("p (h d) -> p h d", h=HG),
                        bias_v[:].rearrange("p (h d) -> p h d", h=HG))
                # K^T and Q^T columns for this 512-chunk of t
                for j in range(JT):
                    ps = ps1.tile([P, 512], f32, tag="ps")
                    for c in range(KT):
                        nc.tensor.matmul(
                            ps[:], wk_t[:, c, j * P:(j + 1) * P],
                            xc[:, c], start=(c == 0), stop=(c == KT - 1))
                    nc.vector.tensor_scalar_add(
                        Kt[:, j, qc * 512:(qc + 1) * 512], ps[:], bk_t[:, j:j + 1])
                for j in range(JT):
                    ps = ps1.tile([P, 512], f32, tag="ps")
                    for c in range(KT):
                        nc.tensor.matmul(
                            ps[:], wq_t[:, c, j * P:(j + 1) * P],
                            xc[:, c], start=(c == 0), stop=(c == KT - 1))
                    nc.vector.tensor_scalar_add(
                        Qt[:, j, qc * 512:(qc + 1) * 512], ps[:], bq_t[:, j:j + 1])

        # ---------------- phase 2: attention ------------------------------
        p_y = es.enter_context(tc.tile_pool(name=pfx + "y", bufs=1))
        Yt = p_y.tile([P, JT, T], f32r)         # y^T (normalized), [j-dim, T]
        p_wp = es.enter_context(tc.tile_pool(name=pfx + "wp", bufs=1))
        wp_t = p_wp.tile([P, JT, C], f32r)
        nc.sync.dma_start(out=wp_t[:], in_=wp[:, :])
        with ExitStack() as es2:
            if 2 not in phases:
                raise _SkipRest
            p_pt = es2.enter_context(tc.tile_pool(name=pfx + "pt", bufs=3))
            p_tmp = es2.enter_context(tc.tile_pool(name=pfx + "ytmp", bufs=2))
            p_dn = es2.enter_context(tc.tile_pool(name=pfx + "dn", bufs=2))
            ps_s = es2.enter_context(
                tc.tile_pool(name=pfx + "ps_s", bufs=2, space="PSUM"))
            ps_y = es2.enter_context(
                tc.tile_pool(name=pfx + "ps_y", bufs=2, space="PSUM"))
            for h in range(HG):
                _attention_head(nc, h, Kt, Qt, V1, Yt, mask_ut, ones65,
                                p_pt, p_tmp, p_dn, ps_s, ps_y)

        # ---------------- phase 3: output projection ----------------------
        if 3 not in phases:
            raise _SkipRest
        with ExitStack() as es3:
            p_o = es3.enter_context(tc.tile_pool(name=pfx + "o", bufs=4))
            ps3 = es3.enter_context(
                tc.tile_pool(name=pfx + "ps3", bufs=4, space="PSUM"))
            for tb in range(TB):
                for ec in range(2):
                    ps = ps3.tile([P, 512], f32, tag="ps")
                    for kt in range(JT):
                        nc.tensor.matmul(
                            ps[:], Yt[:, kt, tb * P:(tb + 1) * P],
                            wp_t[:, kt, ec * 512:(ec + 1) * 512],
                            start=(kt == 0), stop=(kt == JT - 1))
                    o_sb = p_o.tile([P, 512], f32, tag="o")
                    if ec == 0:
                        nc.scalar.copy(o_sb[:], ps[:])
                        nc.scalar.dma_start(out=out[tb, ec], in_=o_sb[:])
                    else:
                        nc.vector.tensor_copy(o_sb[:], ps[:])
                        nc.gpsimd.dma_start(out=out[tb, ec], in_=o_sb[:])


class _SkipRest(Exception):
    pass


def build_program(reps=1, loop_reps=None, phases=(1, 2, 3)):
    nc = bacc.Bacc(None, target_bir_lowering=False)
    xq = nc.declare_dram_parameter("xq", [QC, P, KT, 512], f32r, isOutput=False)
    wq = nc.declare_dram_parameter("wq", [P, KT, DG], f32r, isOutput=False)
    wk = nc.declare_dram_parameter("wk", [P, KT, DG], f32r, isOutput=False)
    wv = nc.declare_dram_parameter("wv", [P, KT, DG], f32r, isOutput=False)
    wp = nc.declare_dram_parameter("wp", [P, JT, C], f32r, isOutput=False)
    bq = nc.declare_dram_parameter("bq", [P, JT], f32, isOutput=False)
    bk = nc.declare_dram_parameter("bk", [P, JT], f32, isOutput=False)
    bv = nc.declare_dram_parameter("bv", [DG], f32r, isOutput=False)
    out = nc.declare_dram_parameter("out", [TB, 2, P, 512], f32, isOutput=True)

    with tile.TileContext(nc) as tc:
        with nc.allow_low_precision(reason="fp32r attention kernel"):
            if loop_reps is not None:
                with tc.For_i(0, loop_reps, 1):
                    try:
                        _emit(nc, tc, 0, xq, wq, wk, wv, wp, bq, bk, bv, out,
                              phases=phases)
                    except _SkipRest:
                        pass
            else:
                for rep in range(reps):
                    try:
                        _emit(nc, tc, rep, xq, wq, wk, wv, wp, bq, bk, bv, out,
                              phases=phases)
                    except _SkipRest:
                        pass
    nc.compile()
    return nc


_PROGRAMS = {}


def _get_program(reps=1):
    if reps not in _PROGRAMS:
        _PROGRAMS[reps] = build_program(reps)
    return _PROGRAMS[reps]


def make_in_maps(x, W_attn, b_attn, W_proj):
    x = np.asarray(x, dtype=np.float32)
    W_attn = np.asarray(W_attn, dtype=np.float32)
    b_attn = np.asarray(b_attn, dtype=np.float32)
    W_proj = np.asarray(W_proj, dtype=np.float32)

    def tile_w(w):  # [C, DG] -> [P, KT, DG]
        return np.ascontiguousarray(w.reshape(KT, P, DG).transpose(1, 0, 2))

    in_maps = []
    for c in range(8):
        b, g = divmod(c, 2)
        sl = slice(g * DG, (g + 1) * DG)
        xT = x[b].T  # [C, T]
        in_maps.append({
            "xq": np.ascontiguousarray(
                xT.reshape(KT, P, QC, 512).transpose(2, 1, 0, 3)),
            "wq": tile_w(W_attn[:, 0 * C:1 * C][:, sl]),
            "wk": tile_w(W_attn[:, 1 * C:2 * C][:, sl]),
            "wv": tile_w(W_attn[:, 2 * C:3 * C][:, sl]),
            "wp": np.ascontiguousarray(
                W_proj[sl, :].reshape(JT, P, C).transpose(1, 0, 2)),
            "bq": np.ascontiguousarray(b_attn[0 * C:1 * C][sl].reshape(JT, P).T),
            "bk": np.ascontiguousarray(b_attn[1 * C:2 * C][sl].reshape(JT, P).T),
            "bv": np.ascontiguousarray(b_attn[2 * C:3 * C][sl]),
        })
    return in_maps


def kernel(x, W_attn, b_attn, W_proj, b_proj, _reps=1):
    nc = _get_program(_reps)
    in_maps = make_in_maps(x, W_attn, b_attn, W_proj)
    res = run_bass_kernel_spmd(nc, in_maps, core_ids=list(range(8)))
    b_proj = np.asarray(b_proj, dtype=np.float32)
    out = np.empty((4, T, C), dtype=np.float32)
    for b in range(4):
        o0 = res.results[2 * b]["out"].transpose(0, 2, 1, 3).reshape(T, C)
        o1 = res.results[2 * b + 1]["out"].transpose(0, 2, 1, 3).reshape(T, C)
        out[b] = o0 + o1 + b_proj
    return out



# revision 4
# speedup vs baseline: 1.1486x; 1.1486x over previous
"""Causal self-attention (B=4, T=2048, C=1024, H=16) on 8 trn2 NeuronCores.

Sharding: data-parallel over batch (4) x tensor-parallel over heads (2 groups
of 8). Core c handles batch c//2, head-group c%2. Each core computes its
partial output projection (W_proj rows of its heads); the host sums the two
head-group partials per batch and adds b_proj.

Per-core kernel (all matmuls fp32r = full PE rate, ~1e-4 rel err):
  phase 1: QKV projection from streamed x^T chunks. Q^T,K^T stored
           [head-dim, T] (j on partitions), V stored [T, heads, 65] with a
           ones column (65th) so the attention AV matmul produces the softmax
           denominator for free.
  phase 2: attention per head, per 1024-wide q-half, per key-block kb of 128:
           S^T[k,q] = K^T_blk.T @ Q^T (causal: only q >= kb*128), exp on ACT
           (scale=1/sqrt(64) folded in), diag-block mask on DVE, then
           y^T[65, q] += V1_kb.T @ P^T_kb accumulated in PSUM over kb.
           Softmax division after the kb loop: reciprocal of the denominator
           row, broadcast to 128 partitions via a K=1 matmul into the second
           y-psum slot, multiply + evict on DVE.
  phase 3: partial out[t, e] = sum_j y^T[j, t] * W_proj[j, e].

All inputs arrive host-pre-tiled in SBUF layout so every load DMA is a
contiguous DRAM read with >=4KB-per-partition descriptors.
"""

from contextlib import ExitStack

import numpy as np

import concourse.bass as bass
import concourse.mybir as mybir
import concourse.tile as tile
from concourse import bacc
from concourse.bass_utils import run_bass_kernel_spmd
from concourse.masks import make_upper_triangular

P = 128
T = 2048
C = 1024
HG = 8          # heads per core
D = 64
DG = HG * D     # 512
KT = C // P     # 8 contraction tiles for the qkv projection
JT = DG // P    # 4 row-tiles of Q^T/K^T (2 heads each)
TB = T // P     # 16 t/key blocks
QC = T // 512   # 4 512-wide column chunks
HQ = T // 1024  # 2 q-halves in attention
f32 = mybir.dt.float32
f32r = mybir.dt.float32r
EXP = mybir.ActivationFunctionType.Exp


class _Piece:
    """One (unit, kb) attention piece: S-matmul + exp + mask, then (later,
    after LOOKAHEAD more pieces have issued their S) the AV accumulate."""

    def __init__(self, unit, kb):
        self.unit = unit
        self.kb = kb
        self.pt = None
        self.q0 = None


class _Unit:
    """One (head, q-half) softmax unit; owns a y^T PSUM accumulator."""

    def __init__(self, h, half):
        self.h = h
        self.half = half
        self.j, self.hp = divmod(h, 2)
        self.pb = self.hp * 64
        self.qlo, self.qhi = half * 1024, (half + 1) * 1024
        self.kmax = self.qhi // P
        self.yt_ps = None
        self.dn = None


def _phase2(nc, Kt, Qt, V1, Yt, mask_ut, ones65, p_pt, p_tmp, p_dn, ps_s, ps_y,
            extra_pe=None):
    """Flat software-pipelined emission over all (head, half) units.

    Per piece: S (PE) -> exp (ACT) -> diag mask (DVE) -> AV (PE), with the AV
    deferred LOOKAHEAD pieces so PE never stalls on ACT. Unit epilogues
    (reciprocal -> broadcast matmul -> scale+evict) are deferred ~2 pieces
    into the next unit. `extra_pe` maps piece ordinals to emitters for
    independent PE work (e.g. phase-3 tiles) injected into the stream.
    """
    LOOK = 2

    def emit_S(pc):
        u = pc.unit
        if u.yt_ps is None:
            u.yt_ps = ps_y.tile([65, 1024], f32, tag="y")
        q0 = max(pc.kb * P, u.qlo)
        plen = u.qhi - q0
        s_ps = ps_s.tile([P, 1024], f32, tag="s")
        for o in range(0, plen, 512):
            clen = min(512, plen - o)
            nc.tensor.matmul(
                s_ps[:, o:o + clen],
                Kt[u.pb:u.pb + 64, u.j, pc.kb * P:(pc.kb + 1) * P],
                Qt[u.pb:u.pb + 64, u.j, q0 + o:q0 + o + clen],
                start=True, stop=True)
        pt = p_pt.tile([P, 1024], f32r, tag="pt")
        nc.scalar.activation(pt[:, :plen], s_ps[:, :plen], EXP, scale=0.125)
        if q0 == pc.kb * P:
            # causal mask inside the diagonal 128x128 block
            nc.vector.tensor_mul(pt[:, 0:P], pt[:, 0:P], mask_ut[:])
        pc.pt, pc.q0 = pt, q0

    def emit_A(pc):
        u = pc.unit
        c_off = pc.q0
        while c_off < u.qhi:
            cell = c_off // 512
            c_end = (cell + 1) * 512
            nc.tensor.matmul(
                u.yt_ps[0:65, c_off - u.qlo:c_end - u.qlo],
                V1[:, pc.kb, u.h],
                pc.pt[:, c_off - pc.q0:c_end - pc.q0],
                start=(pc.kb == 0), stop=(pc.kb == 4 * cell + 3))
            c_off = c_end

    def emit_recip(u):
        u.dn = p_dn.tile([65, 1024], f32r, tag="dn")
        with nc.allow_low_precision(reason="softmax 1/denom"):
            nc.vector.reciprocal(u.dn[64:65, :], u.yt_ps[64:65, :])

    def emit_epi(u):
        bc_ps = ps_s.tile([P, 1024], f32, tag="s")
        for m in range(2):
            sl = slice(m * 512, (m + 1) * 512)
            nc.tensor.matmul(bc_ps[:, sl], ones65[64:65, :], u.dn[64:65, sl],
                             start=True, stop=True)
        # only one non-scalar operand may live in PSUM per DVE op: move the
        # broadcast row block to SBUF, then scale yt_ps (PSUM) against it
        bc_sb = p_tmp.tile([64, 1024], f32r, tag="bc")
        nc.vector.tensor_copy(bc_sb[:], bc_ps[0:64, :])
        sl_t = slice(u.qlo, u.qhi)
        if u.hp == 0:
            nc.vector.tensor_mul(Yt[0:64, u.j, sl_t], u.yt_ps[0:64, :],
                                 bc_sb[:])
        else:
            y_tmp = p_tmp.tile([64, 1024], f32r, tag="yt")
            nc.vector.tensor_mul(y_tmp[:], u.yt_ps[0:64, :], bc_sb[:])
            nc.gpsimd.dma_start(out=Yt[64:128, u.j, sl_t], in_=y_tmp[:])

    pieces = []
    for half in range(HQ):
        for h in range(HG):
            u = _Unit(h, half)
            for kb in range(u.kmax):
                pieces.append(_Piece(u, kb))

    extra_pe = extra_pe or {}
    aq = []          # pieces whose S is emitted but A is not
    epiq = []        # [countdown, unit] epilogues awaiting emission
    n = len(pieces)
    for i in range(n + LOOK):
        if i < n:
            emit_S(pieces[i])
        for e in epiq:
            e[0] -= 1
        while epiq and epiq[0][0] <= 0:
            emit_epi(epiq.pop(0)[1])
        if i >= LOOK:
            pc = pieces[i - LOOK]
            emit_A(pc)
            if pc.kb == pc.unit.kmax - 1:
                emit_recip(pc.unit)
                epiq.append([LOOK, pc.unit])
        ex = extra_pe.pop(i, None)
        if ex is not None:
            ex()
    while epiq:
        emit_epi(epiq.pop(0)[1])


def _emit(nc, tc, rep, xq, wq, wk, wv, wp, bq, bk, bv, out, phases=(1, 2, 3)):
    with ExitStack() as es:
        pfx = f"r{rep}_"
        p_const = es.enter_context(tc.tile_pool(name=pfx + "const", bufs=1))
        mask32 = p_const.tile([P, P], f32)
        make_upper_triangular(nc, mask32[:], val=1.0, diag=True)
        mask_ut = p_const.tile([P, P], f32r)
        nc.vector.tensor_copy(mask_ut[:], mask32[:])
        ones32 = p_const.tile([P, P], f32)
        nc.gpsimd.memset(ones32[:], 1.0)
        ones65 = p_const.tile([65, P], f32r)
        nc.vector.tensor_copy(ones65[:], ones32[0:65, :])
        # keep ACT in the body even in phase-bisected builds (an engine with
        # zero instructions hangs the For_i back-edge barrier)
        act_dummy = p_const.tile([1, 2], f32)
        nc.scalar.copy(act_dummy[:], ones32[0:1, 0:2])

        # persistent tensors
        p_qkv = es.enter_context(tc.tile_pool(name=pfx + "qkv", bufs=1))
        V1 = p_qkv.tile([P, TB, HG, 65], f32r)  # V with ones column, [t, h, d|1]
        Kt = p_qkv.tile([P, JT, T], f32r)       # K^T: [j-dim, T]
        Qt = p_qkv.tile([P, JT, T], f32r)       # Q^T
        # ---------------- phase 1: qkv projection ------------------------
        with ExitStack() as es1:
            p_w1 = es1.enter_context(tc.tile_pool(name=pfx + "w1", bufs=1))
            p_x = es1.enter_context(tc.tile_pool(name=pfx + "x", bufs=2))
            ps1 = es1.enter_context(
                tc.tile_pool(name=pfx + "ps1", bufs=4, space="PSUM"))

            # wv + first x chunk first (first matmuls need them)
            wv_t = p_w1.tile([P, KT, DG], f32r)
            nc.sync.dma_start(out=wv_t[:, 0:2], in_=wv[:, 0:2])
            nc.sync.dma_start(out=wv_t[:, 2:KT], in_=wv[:, 2:KT])
            xc0 = p_x.tile([P, KT, 512], f32r, tag="xc")
            nc.sync.dma_start(out=xc0[:, 0:2], in_=xq[0, :, 0:2])
            nc.sync.dma_start(out=xc0[:, 2:KT], in_=xq[0, :, 2:KT])
            bv_row = p_w1.tile([1, DG], f32r)
            nc.sync.dma_start(out=bv_row[:],
                              in_=bv.rearrange("(o n) -> o n", o=1))
            wk_t = p_w1.tile([P, KT, DG], f32r)
            nc.sync.dma_start(out=wk_t[:], in_=wk[:, :])
            wq_t = p_w1.tile([P, KT, DG], f32r)
            nc.sync.dma_start(out=wq_t[:], in_=wq[:, :])
            bq_t = p_const.tile([P, JT], f32)
            nc.sync.dma_start(out=bq_t[:], in_=bq[:, :])
            bk_t = p_const.tile([P, JT], f32)
            nc.sync.dma_start(out=bk_t[:], in_=bk[:, :])

            # broadcast b_v to 128 partitions (K=1 matmul with a ones row)
            bv_ps = ps1.tile([P, DG], f32, tag="ps")
            nc.tensor.matmul(bv_ps[:], ones65[0:1, :], bv_row[:],
                             start=True, stop=True)
            bias_v = p_w1.tile([P, DG], f32)
            nc.vector.tensor_copy(bias_v[:], bv_ps[:])
            # ones column of V1 (65th dim entry per head)
            nc.vector.tensor_copy(
                V1[:, :, :, 64],
                ones32[:, :].rearrange("p (a b) -> p a b", a=TB))

            for qc in range(QC):
                if qc == 0:
                    xc = xc0
                else:
                    xc = p_x.tile([P, KT, 512], f32r, tag="xc")
                    nc.sync.dma_start(out=xc[:, 0:2], in_=xq[qc, :, 0:2])
                    nc.sync.dma_start(out=xc[:, 2:KT], in_=xq[qc, :, 2:KT])
                # V rows for these 4 t-blocks: out[t, j] = x @ Wv
                for t4 in range(4):
                    tb = qc * 4 + t4
                    ps = ps1.tile([P, DG], f32, tag="ps")
                    for c in range(KT):
                        nc.tensor.matmul(
                            ps[:], xc[:, c, t4 * P:(t4 + 1) * P],
                            wv_t[:, c], start=(c == 0), stop=(c == KT - 1))
                    nc.vector.tensor_add(
                        V1[:, tb, :, 0:64],
                        ps[:].rearrange("p (h d) -> p h d", h=HG),
                        bias_v[:].rearrange("p (h d) -> p h d", h=HG))
                # K^T and Q^T columns for this 512-chunk of t
                for j in range(JT):
                    ps = ps1.tile([P, 512], f32, tag="ps")
                    for c in range(KT):
                        nc.tensor.matmul(
                            ps[:], wk_t[:, c, j * P:(j + 1) * P],
                            xc[:, c], start=(c == 0), stop=(c == KT - 1))
                    nc.vector.tensor_scalar_add(
                        Kt[:, j, qc * 512:(qc + 1) * 512], ps[:], bk_t[:, j:j + 1])
                for j in range(JT):
                    ps = ps1.tile([P, 512], f32, tag="ps")
                    for c in range(KT):
                        nc.tensor.matmul(
                            ps[:], wq_t[:, c, j * P:(j + 1) * P],
                            xc[:, c], start=(c == 0), stop=(c == KT - 1))
                    nc.vector.tensor_scalar_add(
                        Qt[:, j, qc * 512:(qc + 1) * 512], ps[:], bq_t[:, j:j + 1])

        # ---------------- phase 2: attention ------------------------------
        p_y = es.enter_context(tc.tile_pool(name=pfx + "y", bufs=1))
        Yt = p_y.tile([P, JT, T], f32r)         # y^T (normalized), [j-dim, T]
        p_wp = es.enter_context(tc.tile_pool(name=pfx + "wp", bufs=1))
        wp_t = p_wp.tile([P, JT, C], f32r)
        nc.sync.dma_start(out=wp_t[:], in_=wp[:, :])
        with ExitStack() as es2:
            if 2 not in phases:
                raise _SkipRest
            p_pt = es2.enter_context(tc.tile_pool(name=pfx + "pt", bufs=3))
            p_tmp = es2.enter_context(tc.tile_pool(name=pfx + "ytmp", bufs=2))
            p_dn = es2.enter_context(tc.tile_pool(name=pfx + "dn", bufs=2))
            ps_s = es2.enter_context(
                tc.tile_pool(name=pfx + "ps_s", bufs=2, space="PSUM"))
            ps_y = es2.enter_context(
                tc.tile_pool(name=pfx + "ps_y", bufs=2, space="PSUM"))
            _phase2(nc, Kt, Qt, V1, Yt, mask_ut, ones65,
                    p_pt, p_tmp, p_dn, ps_s, ps_y)

        # ---------------- phase 3: output projection ----------------------
        if 3 not in phases:
            raise _SkipRest
        with ExitStack() as es3:
            p_o = es3.enter_context(tc.tile_pool(name=pfx + "o", bufs=4))
            ps3 = es3.enter_context(
                tc.tile_pool(name=pfx + "ps3", bufs=4, space="PSUM"))
            for tb in range(TB):
                for ec in range(2):
                    ps = ps3.tile([P, 512], f32, tag="ps")
                    for kt in range(JT):
                        nc.tensor.matmul(
                            ps[:], Yt[:, kt, tb * P:(tb + 1) * P],
                            wp_t[:, kt, ec * 512:(ec + 1) * 512],
                            start=(kt == 0), stop=(kt == JT - 1))
                    o_sb = p_o.tile([P, 512], f32, tag="o")
                    if ec == 0:
                        nc.scalar.copy(o_sb[:], ps[:])
                        nc.scalar.dma_start(out=out[tb, ec], in_=o_sb[:])
                    else:
                        nc.vector.tensor_copy(o_sb[:], ps[:])
                        nc.gpsimd.dma_start(out=out[tb, ec], in_=o_sb[:])


class _SkipRest(Exception):
    pass


def build_program(reps=1, loop_reps=None, phases=(1, 2, 3)):
    nc = bacc.Bacc(None, target_bir_lowering=False)
    xq = nc.declare_dram_parameter("xq", [QC, P, KT, 512], f32r, isOutput=False)
    wq = nc.declare_dram_parameter("wq", [P, KT, DG], f32r, isOutput=False)
    wk = nc.declare_dram_parameter("wk", [P, KT, DG], f32r, isOutput=False)
    wv = nc.declare_dram_parameter("wv", [P, KT, DG], f32r, isOutput=False)
    wp = nc.declare_dram_parameter("wp", [P, JT, C], f32r, isOutput=False)
    bq = nc.declare_dram_parameter("bq", [P, JT], f32, isOutput=False)
    bk = nc.declare_dram_parameter("bk", [P, JT], f32, isOutput=False)
    bv = nc.declare_dram_parameter("bv", [DG], f32r, isOutput=False)
    out = nc.declare_dram_parameter("out", [TB, 2, P, 512], f32, isOutput=True)

    with tile.TileContext(nc) as tc:
        with nc.allow_low_precision(reason="fp32r attention kernel"):
            if loop_reps is not None:
                with tc.For_i(0, loop_reps, 1):
                    try:
                        _emit(nc, tc, 0, xq, wq, wk, wv, wp, bq, bk, bv, out,
                              phases=phases)
                    except _SkipRest:
                        pass
            else:
                for rep in range(reps):
                    try:
                        _emit(nc, tc, rep, xq, wq, wk, wv, wp, bq, bk, bv, out,
                              phases=phases)
                    except _SkipRest:
                        pass
    nc.compile()
    return nc


_PROGRAMS = {}


def _get_program(reps=1):
    if reps not in _PROGRAMS:
        _PROGRAMS[reps] = build_program(reps)
    return _PROGRAMS[reps]


def make_in_maps(x, W_attn, b_attn, W_proj):
    x = np.asarray(x, dtype=np.float32)
    W_attn = np.asarray(W_attn, dtype=np.float32)
    b_attn = np.asarray(b_attn, dtype=np.float32)
    W_proj = np.asarray(W_proj, dtype=np.float32)

    def tile_w(w):  # [C, DG] -> [P, KT, DG]
        return np.ascontiguousarray(w.reshape(KT, P, DG).transpose(1, 0, 2))

    in_maps = []
    for c in range(8):
        b, g = divmod(c, 2)
        sl = slice(g * DG, (g + 1) * DG)
        xT = x[b].T  # [C, T]
        in_maps.append({
            "xq": np.ascontiguousarray(
                xT.reshape(KT, P, QC, 512).transpose(2, 1, 0, 3)),
            "wq": tile_w(W_attn[:, 0 * C:1 * C][:, sl]),
            "wk": tile_w(W_attn[:, 1 * C:2 * C][:, sl]),
            "wv": tile_w(W_attn[:, 2 * C:3 * C][:, sl]),
            "wp": np.ascontiguousarray(
                W_proj[sl, :].reshape(JT, P, C).transpose(1, 0, 2)),
            "bq": np.ascontiguousarray(b_attn[0 * C:1 * C][sl].reshape(JT, P).T),
            "bk": np.ascontiguousarray(b_attn[1 * C:2 * C][sl].reshape(JT, P).T),
            "bv": np.ascontiguousarray(b_attn[2 * C:3 * C][sl]),
        })
    return in_maps


def kernel(x, W_attn, b_attn, W_proj, b_proj, _reps=1):
    nc = _get_program(_reps)
    in_maps = make_in_maps(x, W_attn, b_attn, W_proj)
    res = run_bass_kernel_spmd(nc, in_maps, core_ids=list(range(8)))
    b_proj = np.asarray(b_proj, dtype=np.float32)
    out = np.empty((4, T, C), dtype=np.float32)
    for b in range(4):
        o0 = res.results[2 * b]["out"].transpose(0, 2, 1, 3).reshape(T, C)
        o1 = res.results[2 * b + 1]["out"].transpose(0, 2, 1, 3).reshape(T, C)
        out[b] = o0 + o1 + b_proj
    return out

